# revision 10
# baseline (speedup 1.0000x reference)
"""Trainium2 Bass kernel for nn_C3S_RegularLoss.

reference:
    xr = x.reshape(B, P, D); xn = xr / ||xr||_2(axis=-1)
    s = mean_b(xn)                     # (P, D)
    corr = s @ s.T                     # (P, P)
    loss = (sum(corr) - 3*trace(corr) + 2P) / 2 * gamma

Reformulated without the corr matrix:
    sum(corr)   = || sum_p s_p ||^2
    trace(corr) = sum_p || s_p ||^2
so with S = sum_b xn (sum, not mean):
    loss = ((||sum_p S_p||^2 - 3*sum(S^2)) / B^2 + 2P) / 2 * gamma

Sharding: data-parallel over the batch dim, 8 cores x 1024 rows.
Each core computes S_partial = sum_b r_b * x_b per part via PE matmuls
(r = 1/||x_part|| as the stationary operand), AllReduce of the (4,2048)
sums, then a tiny replicated tail computes the scalar loss.

Timeline design (per-core, numbers from baseline trace):
  - The HBM load stream is saturated (~334 GB/s) and ends ~108 us; the
    whole game after that is shortening the serial tail:
      finalize last tile -> cc_in_b DMA -> AR2 doorbell -> AR2 (mesh,
      floor ~10us + rank skew) -> tail scalar math -> out.
  - Tile 0 is DMA'd per part so its normalize/matmul chain, the S_a
    copy-out and the AR1 doorbell all fire by ~26 us (baseline: 54 us).
    AR1 then completes ~55 us, warming the collective stream and
    guaranteeing AR2 never queues behind it.
  - Tile 7 is DMA'd per part, and its last part in 4 column chunks.
    Parts 0-2 square-accumulate on ACT under the tail of the DMA
    stream; part 3's chunks go through DVE tensor_tensor_reduce
    (square+accumulate in one pass, chained via the scalar operand) so
    its 1/norm is ready ~1 us after the last bytes land.  Target: AR2
    doorbell ~112 us instead of 124 us.
  - While AR2 runs, the AR1 half of the tail is precomputed (sfa load,
    ones-matmuls into t_ps).  After AR2: sfb load, one DVE add, one DVE
    square-reduce (B2), one ACT square-accum (A), a -3*ones matmul and
    two tiny DVE ops -> out.
"""

import os
import sys

sys.path.insert(0, "/opt/trn_rl_repo")
os.environ.setdefault("MYCRO_LOCAL_CACHE", "1")

import numpy as np

B, F = 8192, 8192
NPARTS = 4
D = F // NPARTS                 # 2048
NCORES = 8
B_CORE = B // NCORES            # 1024
TILE_P = 128
NTILES = B_CORE // TILE_P       # 8
MM_N = 512                      # moving free dim per matmul
NCHUNK = D // MM_N              # 4
HALF = 1                        # tiles in the first (early-AllReduce) group

_cache = {}


def _build(ncores=NCORES, collective=True):
    import concourse.bass as bass  # noqa: F401
    import concourse.mybir as mybir
    from concourse import bacc, tile
    from concourse.tile import add_dep_helper

    f32 = mybir.dt.float32
    bf16 = mybir.dt.bfloat16
    Act = mybir.ActivationFunctionType
    Alu = mybir.AluOpType

    nc = bacc.Bacc("TRN2", num_devices=ncores, debug=False)
    x_t = nc.dram_tensor("x", [B_CORE, F], f32, kind="ExternalInput")
    g_t = nc.dram_tensor("gamma", [1, 1], f32, kind="ExternalInput")
    out_t = nc.dram_tensor("out", [1, 1], f32, kind="ExternalOutput")

    with tile.TileContext(nc) as tc:
        with tc.tile_pool(name="xp", bufs=NTILES) as xp, \
             tc.tile_pool(name="scratch", bufs=2) as scp, \
             tc.tile_pool(name="small", bufs=3) as stp, \
             tc.tile_pool(name="tail", bufs=1) as tlp, \
             tc.tile_pool(name="ps", bufs=1, space="PSUM") as psp, \
             tc.tile_pool(name="dram", bufs=1, space="DRAM") as dram:

            # PSUM accumulators: part p lives at psum partition 32*p
            # (PE col tile_position constraint). Two 4-bank accumulators:
            # tile 0 -> S_a (AllReduce'd at ~26us, done ~55us: warms the
            # collective stream and absorbs rank skew), tiles 1..7 -> S_b
            # (AllReduce right after the DMA stream ends).
            S_a = psp.tile([TILE_P, D], f32, tag="accA")
            S_b = psp.tile([TILE_P, D], f32, tag="accB")
            # init the junk rows (everything besides 0/32/64/96) so the
            # later full-width PSUM->SBUF copies read defined data; DVE
            # is idle here and PSUM writes don't touch the SBUF ports
            # the SWDGE descriptor rings need. One memset per PSUM bank.
            for b in range(NCHUNK):
                nc.vector.memset(S_a[:, b * MM_N:(b + 1) * MM_N], 0.0)
                nc.vector.memset(S_b[:, b * MM_N:(b + 1) * MM_N], 0.0)
            cc_in_a = dram.tile([NPARTS, D], f32)
            cc_out_a = dram.tile([NPARTS, D], f32)
            cc_in_b = dram.tile([NPARTS, D], f32)
            cc_out_b = dram.tile([NPARTS, D], f32)

            prev_sqrt = None
            ar1 = None
            for i in range(NTILES):
                first = i == 0
                last = i == NTILES - 1
                # SWDGE DMA casts fp32 -> bf16 in-flight (free; PE wants
                # bf16 and the loss has ~1e3x precision headroom).
                # First and last tile: split per part so their (exposed)
                # normalize chains start at each part boundary; last
                # part of the last tile further split into 4 chunks so
                # DVE square-accumulate tracks the arriving data.
                xt = xp.tile([TILE_P, F], bf16, tag="xt")
                rows = x_t[i * TILE_P:(i + 1) * TILE_P, :]
                if first or last:
                    for p in range(NPARTS):
                        if last and p == NPARTS - 1:
                            for c in range(NCHUNK):
                                lo = p * D + c * MM_N
                                nc.gpsimd.dma_start(xt[:, lo:lo + MM_N],
                                                    rows[:, lo:lo + MM_N])
                        else:
                            nc.gpsimd.dma_start(xt[:, p * D:(p + 1) * D],
                                                rows[:, p * D:(p + 1) * D])
                else:
                    nc.gpsimd.dma_start(xt[:], rows)

                # sum-of-squares per part on ACT (square + free
                # accumulator). Keeping the big elementwise ops OFF the
                # vector engine matters mid-stream: DVE SBUF reads lock
                # GpSimd out of the port it uses for SWDGE descriptor
                # rings, which stalls the x-tile DMA stream.
                ss = stp.tile([TILE_P, NPARTS], f32, tag="ss")
                sqa = scp.tile([TILE_P, D], bf16, tag="sqa")
                norm = stp.tile([TILE_P, NPARTS], f32, tag="norm")
                r = stp.tile([TILE_P, NPARTS], f32, tag="r")
                r_bf = stp.tile([TILE_P, NPARTS], bf16, tag="r_bf")
                S_ps = S_a if i < HALF else S_b

                def mms_for_part(p, rbf_ap):
                    for j in range(NCHUNK):
                        nc.tensor.matmul(
                            S_ps[32 * p:32 * p + 1, j * MM_N:(j + 1) * MM_N],
                            lhsT=rbf_ap,
                            rhs=xt[:, p * D + j * MM_N:p * D + (j + 1) * MM_N],
                            start=(i == 0 or i == HALF),
                            stop=(i == HALF - 1 or i == NTILES - 1),
                            tile_position=(0, 32 * p))

                if not (first or last):
                    for p in range(NPARTS):
                        a = nc.scalar.activation(
                            sqa[:], xt[:, p * D:(p + 1) * D], Act.Square,
                            accum_out=ss[:, p:p + 1])
                        if p == 0 and prev_sqrt is not None:
                            # pin ACT order: sqrt(i-1) must precede
                            # squares(i), else the scheduler makes r(i-1)
                            # wait on DMA(i)
                            add_dep_helper(
                                a.ins, prev_sqrt.ins, sync=False,
                                reason="sqrt(i-1) before squares(i)")
                    prev_sqrt = nc.scalar.sqrt(norm[:], ss[:])
                    nc.vector.reciprocal(r[:], norm[:])
                    nc.vector.tensor_copy(r_bf[:], r[:])
                    for p in range(NPARTS):
                        mms_for_part(p, r_bf[:, p:p + 1])
                elif first:
                    # per-part chain: square -> sqrt -> recip -> cast ->
                    # matmuls, so S_a (and with it the AR1 doorbell)
                    # completes as early as possible
                    pa = None
                    for p in range(NPARTS):
                        a = nc.scalar.activation(
                            sqa[:], xt[:, p * D:(p + 1) * D], Act.Square,
                            accum_out=ss[:, p:p + 1])
                        if pa is not None:
                            add_dep_helper(a.ins, pa.ins, sync=False,
                                           reason="ACT part order")
                        pa = nc.scalar.sqrt(norm[:, p:p + 1], ss[:, p:p + 1])
                        nc.vector.reciprocal(r[:, p:p + 1], norm[:, p:p + 1])
                        nc.vector.tensor_copy(r_bf[:, p:p + 1], r[:, p:p + 1])
                        mms_for_part(p, r_bf[:, p:p + 1])
                    prev_sqrt = pa
                else:
                    # last tile: parts 0-2 square on ACT (their data
                    # lands while the stream is still running); part 3
                    # squares chunk-by-chunk on DVE tensor_tensor_reduce
                    # (accumulator chained through `scalar`) so the
                    # final 1/norm is ready ~1us after the stream ends.
                    ss3 = stp.tile([TILE_P, NCHUNK], f32, tag="ss3")
                    pa = None
                    for p in range(NPARTS - 1):
                        a = nc.scalar.activation(
                            sqa[:], xt[:, p * D:(p + 1) * D], Act.Square,
                            accum_out=ss[:, p:p + 1])
                        if p == 0 and prev_sqrt is not None:
                            add_dep_helper(a.ins, prev_sqrt.ins, sync=False,
                                           reason="sqrt(i-1) first")
                        if pa is not None:
                            add_dep_helper(a.ins, pa.ins, sync=False,
                                           reason="ACT part order")
                        pa = nc.scalar.sqrt(norm[:, p:p + 1], ss[:, p:p + 1])
                        nc.vector.reciprocal(r[:, p:p + 1], norm[:, p:p + 1])
                        nc.vector.tensor_copy(r_bf[:, p:p + 1], r[:, p:p + 1])
                        mms_for_part(p, r_bf[:, p:p + 1])
                    p3 = NPARTS - 1
                    sq3 = scp.tile([TILE_P, D], bf16, tag="sq3")
                    for c in range(NCHUNK):
                        lo = p3 * D + c * MM_N
                        nc.vector.tensor_mul(
                            sq3[:, c * MM_N:(c + 1) * MM_N],
                            xt[:, lo:lo + MM_N], xt[:, lo:lo + MM_N])
                        nc.vector.tensor_reduce(
                            ss3[:, c:c + 1],
                            sq3[:, c * MM_N:(c + 1) * MM_N],
                            axis=mybir.AxisListType.X, op=Alu.add)
                    ss3m = stp.tile([TILE_P, 3], f32, tag="ss3m")
                    nc.vector.tensor_add(ss3m[:, 0:1], ss3[:, 0:1], ss3[:, 1:2])
                    nc.vector.tensor_add(ss3m[:, 1:2], ss3[:, 2:3], ss3[:, 3:4])
                    nc.vector.tensor_add(ss3m[:, 2:3],
                                         ss3m[:, 0:1], ss3m[:, 1:2])
                    s3 = nc.scalar.sqrt(norm[:, p3:p3 + 1], ss3m[:, 2:3])
                    if pa is not None:
                        add_dep_helper(s3.ins, pa.ins, sync=False,
                                       reason="ACT part order")
                    prev_sqrt = s3
                    nc.vector.reciprocal(r[:, p3:p3 + 1], norm[:, p3:p3 + 1])
                    nc.vector.tensor_copy(r_bf[:, p3:p3 + 1], r[:, p3:p3 + 1])
                    mms_for_part(p3, r_bf[:, p3:p3 + 1])

                if i == HALF - 1:
                    # first-tile partial sums: ship out + AllReduce now,
                    # overlapped with the rest of the DMA stream
                    s_sba = tlp.tile([TILE_P, D], f32, tag="s_sba")
                    nc.vector.tensor_copy(s_sba[:], S_a[:])
                    for p in range(NPARTS):
                        nc.sync.dma_start(cc_in_a[p:p + 1, :],
                                          s_sba[32 * p:32 * p + 1, :])
                    if collective:
                        ar1 = nc.gpsimd.collective_compute(
                            "AllReduce", Alu.add,
                            replica_groups=[list(range(ncores))],
                            ins=[cc_in_a.opt()], outs=[cc_out_a.opt()])
                    else:
                        nc.sync.dma_start(cc_out_a[:], cc_in_a[:])

            # ---- AR1 tail precompute (runs in the idle window while the
            # stream and later AR2 are in flight): load summed first-half
            # as bf16, accumulate sum_p via ones-matmuls into t_ps ----
            sfa = tlp.tile([NPARTS, D], bf16, tag="sfa")
            ld_a = nc.gpsimd.dma_start(sfa[:], cc_out_a[:])
            ones4 = tlp.tile([NPARTS, 1], bf16, tag="ones4")
            nc.vector.memset(ones4[:], 1.0)
            # weights folding A - 3*B2 via two accumulating matmuls
            neg3 = tlp.tile([NPARTS, 1], f32, tag="neg3")
            nc.vector.memset(neg3[:], -3.0)
            one1 = tlp.tile([1, 1], f32, tag="one1")
            nc.vector.memset(one1[:], 1.0)
            g_sb = tlp.tile([1, 1], f32, tag="g_sb")
            nc.sync.dma_start(g_sb[:], g_t[:])

            # t = sum_p S_p accumulates in PSUM: sfa half now (start),
            # sfb half after AR2 (stop) — the sfa+sfb add is off the
            # t critical path entirely
            t_ps = psp.tile([1, D], f32, tag="accA")
            for j in range(NCHUNK):
                nc.tensor.matmul(
                    t_ps[0:1, j * MM_N:(j + 1) * MM_N],
                    lhsT=ones4[:],
                    rhs=sfa[:, j * MM_N:(j + 1) * MM_N],
                    start=True, stop=False)

            # ---- second-group partial sums -> AllReduce over 8 cores ----
            # one full-width PSUM->SBUF copy split across ACT and DVE
            # (rows besides 0/32/64/96 are junk but harmless)
            s_sb = tlp.tile([TILE_P, D], f32, tag="s_sb")
            nc.scalar.copy(s_sb[:, :D // 2], S_b[:, :D // 2])
            nc.vector.tensor_copy(s_sb[:, D // 2:], S_b[:, D // 2:])

            for p in range(NPARTS):
                eng = nc.sync if p % 2 == 0 else nc.scalar
                eng.dma_start(cc_in_b[p:p + 1, :],
                              s_sb[32 * p:32 * p + 1, :])
            ar2 = None
            if collective:
                ar2 = nc.gpsimd.collective_compute(
                    "AllReduce", Alu.add,
                    replica_groups=[list(range(ncores))],
                    ins=[cc_in_b.opt()], outs=[cc_out_b.opt()])
                # keep gpsimd free to fire the AR2 doorbell before it
                # blocks on loading AR2's output
                add_dep_helper(ld_a.ins, ar1.ins, sync=False,
                               reason="AR1 done before sfa load")
            else:
                nc.sync.dma_start(cc_out_b[:], cc_in_b[:])

            sfb = tlp.tile([NPARTS, D], bf16, tag="sfb")
            ld_b = nc.gpsimd.dma_start(sfb[:], cc_out_b[:])
            if ar2 is not None:
                add_dep_helper(ld_b.ins, ar2.ins, sync=False,
                               reason="AR2 doorbell before sfb load")

            # ---- replicated tail: loss scalar ----
            for j in range(NCHUNK):
                nc.tensor.matmul(
                    t_ps[0:1, j * MM_N:(j + 1) * MM_N],
                    lhsT=ones4[:],
                    rhs=sfb[:, j * MM_N:(j + 1) * MM_N],
                    start=False, stop=True)

            # B2 = sum((sfa+sfb)^2) on DVE (mult + reduce) in parallel
            # with A = ||t||^2 on ACT; two accumulating matmuls with
            # lhsT -3*ones / +1 fold them into ab_ps = A - 3*B2.
            sfull = tlp.tile([NPARTS, D], bf16, tag="sfull")
            nc.vector.tensor_add(sfull[:], sfa[:], sfb[:])
            sq_tail = tlp.tile([NPARTS, D], bf16, tag="sq_tail")
            nc.vector.tensor_mul(sq_tail[:], sfull[:], sfull[:])
            ssum = tlp.tile([NPARTS, 1], f32, tag="ssum")
            nc.vector.tensor_reduce(ssum[:], sq_tail[:],
                                    axis=mybir.AxisListType.X, op=Alu.add)

            t_sq = tlp.tile([1, D], f32, tag="t_sq")
            a_sb = tlp.tile([1, 1], f32, tag="a_sb")
            nc.scalar.activation(t_sq[:], t_ps[:], Act.Square,
                                 accum_out=a_sb[:])

            ab_ps = psp.tile([1, 1], f32, tag="accB")
            nc.tensor.matmul(ab_ps[:], lhsT=neg3[:], rhs=ssum[:],
                             start=True, stop=False)
            nc.tensor.matmul(ab_ps[:], lhsT=one1[:], rhs=a_sb[:],
                             start=False, stop=True)

            # loss = ((A - 3*B2) / B^2 + 2P) / 2 * gamma
            l0 = tlp.tile([1, 1], f32, tag="l0")
            nc.vector.tensor_scalar(
                out=l0[:], in0=ab_ps[:],
                scalar1=1.0 / (2.0 * float(B) * float(B)),
                scalar2=float(NPARTS),
                op0=Alu.mult, op1=Alu.add)
            loss = tlp.tile([1, 1], f32, tag="loss")
            nc.vector.tensor_mul(loss[:], l0[:], g_sb[:])
            nc.sync.dma_start(out_t[:], loss[:])

    nc.compile()
    return nc


def _get_nc():
    if "nc" not in _cache:
        _cache["nc"] = _build()
    return _cache["nc"]


def kernel(x, gamma, **run_kwargs):
    from concourse import bass_utils

    x = np.ascontiguousarray(np.asarray(x, dtype=np.float32))
    gamma = np.asarray(gamma, dtype=np.float32).reshape(1, 1)
    assert x.shape == (B, F), x.shape

    nc = _get_nc()
    in_maps = [
        {"x": x[c * B_CORE:(c + 1) * B_CORE], "gamma": gamma}
        for c in range(NCORES)
    ]
    res = bass_utils.run_bass_kernel_spmd(
        nc, in_maps, core_ids=list(range(NCORES)), **run_kwargs)
    out = np.asarray(res.results[0]["out"], dtype=np.float32).reshape(1)
    if run_kwargs.get("trace"):
        _cache["last_results"] = res
    return out


# revision 12
# speedup vs baseline: 1.0847x; 1.0847x over previous
"""Trainium2 Bass kernel for nn_C3S_RegularLoss.

reference:
    xr = x.reshape(B, P, D); xn = xr / ||xr||_2(axis=-1)
    s = mean_b(xn)                     # (P, D)
    corr = s @ s.T                     # (P, P)
    loss = (sum(corr) - 3*trace(corr) + 2P) / 2 * gamma

Reformulated without the corr matrix:
    sum(corr)   = || sum_p s_p ||^2
    trace(corr) = sum_p || s_p ||^2
so with S = sum_b xn (sum, not mean):
    loss = ((||sum_p S_p||^2 - 3*sum(S^2)) / B^2 + 2P) / 2 * gamma

Sharding: data-parallel over the batch dim, 8 cores x 1024 rows.
Each core computes S_partial = sum_b r_b * x_b per part via PE matmuls
(r = 1/||x_part|| as the stationary operand), AllReduce of the (4,2048)
sums, then a tiny replicated tail computes the scalar loss.

Timeline design (from trace analysis; per-core, all times us):
  - The HBM load stream saturates (~334 GB/s) and ends ~108; everything
    after is the serial tail: finalize tile 7 -> cc_in_b DMA -> AR2
    doorbell -> AR2 mesh (floor ~10 + rank skew) -> scalar tail -> out.
  - ACT does the per-part sum-of-squares (2us per part).  With one DMA
    per tile, tile i's squares only start when the whole tile lands, so
    ACT runs ~10us behind the stream and tiles 6+7 stack up ~17us of
    ACT work after the stream ends.  Fixes: tile 6 offloads its last
    part to DVE (mult+reduce), tile 7 is DMA'd [p0p1][p2][p3c0][p3c1]
    with p3's two chunks squared on DVE as they land.  Target: AR2
    doorbell ~115 instead of 124-126.
  - Tile 0 is DMA'd in two halves and its normalize chain runs
    per-part so S_a, its copy-out (pinned early on the DVE queue) and
    the AR1 doorbell fire by ~29 (was 51-54).  AR1 completes ~70-90,
    warming the collective stream, absorbing rank skew, and leaving
    the idle window for the sfa half of the tail.
  - Both ACT table sets (square, sqrt) are pre-loaded with dummy ops
    so the first sqrt isn't gated by a mid-chain ~1.3us table load.
  - Tail after AR2: sfb load, DVE add/mult, B2 via PE ones-matmuls
    (a [1,512] PSUM accumulation) + one short ACT accumulate instead
    of DVE tensor_reduce (2.3us); A and B2 fold into A-3*B2 through
    two accumulating matmuls.
"""

import os
import sys

sys.path.insert(0, "/opt/trn_rl_repo")
os.environ.setdefault("MYCRO_LOCAL_CACHE", "1")

import numpy as np

B, F = 8192, 8192
NPARTS = 4
D = F // NPARTS                 # 2048
NCORES = 8
B_CORE = B // NCORES            # 1024
TILE_P = 128
NTILES = B_CORE // TILE_P       # 8
MM_N = 512                      # moving free dim per matmul
NCHUNK = D // MM_N              # 4
HALF = 1                        # tiles in the first (early-AllReduce) group

_cache = {}


def _build(ncores=NCORES, collective=True):
    import concourse.bass as bass  # noqa: F401
    import concourse.mybir as mybir
    from concourse import bacc, tile
    from concourse.tile import add_dep_helper

    f32 = mybir.dt.float32
    bf16 = mybir.dt.bfloat16
    Act = mybir.ActivationFunctionType
    Alu = mybir.AluOpType
    AxX = mybir.AxisListType.X

    nc = bacc.Bacc("TRN2", num_devices=ncores, debug=False)
    x_t = nc.dram_tensor("x", [B_CORE, F], f32, kind="ExternalInput")
    g_t = nc.dram_tensor("gamma", [1, 1], f32, kind="ExternalInput")
    out_t = nc.dram_tensor("out", [1, 1], f32, kind="ExternalOutput")

    with tile.TileContext(nc) as tc:
        with tc.tile_pool(name="xp", bufs=NTILES) as xp, \
             tc.tile_pool(name="scratch", bufs=2) as scp, \
             tc.tile_pool(name="small", bufs=3) as stp, \
             tc.tile_pool(name="tail", bufs=1) as tlp, \
             tc.tile_pool(name="ps", bufs=1, space="PSUM") as psp, \
             tc.tile_pool(name="dram", bufs=1, space="DRAM") as dram:

            # PSUM accumulators: part p lives at psum partition 32*p
            # (PE col tile_position constraint). Two 4-bank accumulators:
            # tile 0 -> S_a (AllReduce'd at ~29us: warms the collective
            # stream and absorbs rank skew), tiles 1..7 -> S_b
            # (AllReduce right after the DMA stream ends).
            S_a = psp.tile([TILE_P, D], f32, tag="accA")
            S_b = psp.tile([TILE_P, D], f32, tag="accB")
            # init the junk rows (everything besides 0/32/64/96) so the
            # later full-width PSUM->SBUF copies read defined data; DVE
            # is idle here and PSUM writes don't touch the SBUF ports
            # the SWDGE descriptor rings need. One memset per PSUM bank.
            for bk in range(NCHUNK):
                nc.vector.memset(S_a[:, bk * MM_N:(bk + 1) * MM_N], 0.0)
                nc.vector.memset(S_b[:, bk * MM_N:(bk + 1) * MM_N], 0.0)
            cc_in_a = dram.tile([NPARTS, D], f32)
            cc_out_a = dram.tile([NPARTS, D], f32)
            cc_in_b = dram.tile([NPARTS, D], f32)
            cc_out_b = dram.tile([NPARTS, D], f32)

            # pre-load both ACT table sets (square, sqrt) with dummy ops
            # so tile 0's chain isn't gated by mid-chain table loads
            warm = tlp.tile([1, 2], f32, tag="warm")
            nc.vector.memset(warm[:], 1.0)
            warm2 = tlp.tile([1, 2], f32, tag="warm2")
            nc.scalar.activation(warm2[:, 0:1], warm[:, 0:1], Act.Square)
            nc.scalar.sqrt(warm2[:, 1:2], warm[:, 1:2])

            prev_sqrt = None
            ar1 = None
            cp_a = None
            for i in range(NTILES):
                first = i == 0
                last = i == NTILES - 1
                # SWDGE DMA casts fp32 -> bf16 in-flight (free; PE wants
                # bf16 and the loss has ~1e3x precision headroom).
                xt = xp.tile([TILE_P, F], bf16, tag="xt")
                rows = x_t[i * TILE_P:(i + 1) * TILE_P, :]
                if first:
                    # two halves so the per-part normalize chain (and
                    # with it the AR1 doorbell) starts ~6us earlier
                    for h in range(2):
                        lo = h * (F // 2)
                        nc.gpsimd.dma_start(xt[:, lo:lo + F // 2],
                                            rows[:, lo:lo + F // 2])
                elif last:
                    # [p0p1][p2][p3c0][p3c1]: p3's chunks square on DVE
                    # as they land, so 1/norm is ready ~1us after the
                    # stream ends
                    nc.gpsimd.dma_start(xt[:, :2 * D], rows[:, :2 * D])
                    nc.gpsimd.dma_start(xt[:, 2 * D:3 * D],
                                        rows[:, 2 * D:3 * D])
                    nc.gpsimd.dma_start(xt[:, 3 * D:3 * D + D // 2],
                                        rows[:, 3 * D:3 * D + D // 2])
                    nc.gpsimd.dma_start(xt[:, 3 * D + D // 2:],
                                        rows[:, 3 * D + D // 2:])
                else:
                    nc.gpsimd.dma_start(xt[:], rows)

                # sum-of-squares per part on ACT (square + free
                # accumulator). Keeping the big elementwise ops OFF the
                # vector engine matters mid-stream: DVE SBUF reads lock
                # GpSimd out of the port it uses for SWDGE descriptor
                # rings, which stalls the x-tile DMA stream. (Late in
                # the stream all descriptors are long emitted, so tiles
                # 6/7 can use DVE freely.)
                ss = stp.tile([TILE_P, NPARTS], f32, tag="ss")
                sqa = scp.tile([TILE_P, D], bf16, tag="sqa")
                norm = stp.tile([TILE_P, NPARTS], f32, tag="norm")
                r = stp.tile([TILE_P, NPARTS], f32, tag="r")
                r_bf = stp.tile([TILE_P, NPARTS], bf16, tag="r_bf")
                S_ps = S_a if i < HALF else S_b

                def mms_for_part(p, rbf_ap):
                    for j in range(NCHUNK):
                        nc.tensor.matmul(
                            S_ps[32 * p:32 * p + 1, j * MM_N:(j + 1) * MM_N],
                            lhsT=rbf_ap,
                            rhs=xt[:, p * D + j * MM_N:p * D + (j + 1) * MM_N],
                            start=(i == 0 or i == HALF),
                            stop=(i == HALF - 1 or i == NTILES - 1),
                            tile_position=(0, 32 * p))

                if first:
                    # per-part chain: square -> sqrt -> recip -> cast ->
                    # matmuls, so S_a completes as early as possible
                    pa = None
                    for p in range(NPARTS):
                        a = nc.scalar.activation(
                            sqa[:], xt[:, p * D:(p + 1) * D], Act.Square,
                            accum_out=ss[:, p:p + 1])
                        if pa is not None:
                            add_dep_helper(a.ins, pa.ins, sync=False,
                                           reason="ACT part order")
                        pa = nc.scalar.sqrt(norm[:, p:p + 1], ss[:, p:p + 1])
                        nc.vector.reciprocal(r[:, p:p + 1], norm[:, p:p + 1])
                        nc.vector.tensor_copy(r_bf[:, p:p + 1], r[:, p:p + 1])
                        mms_for_part(p, r_bf[:, p:p + 1])
                    prev_sqrt = pa
                elif not last:
                    dve_p3 = i == NTILES - 2
                    for p in range(NPARTS - 1 if dve_p3 else NPARTS):
                        a = nc.scalar.activation(
                            sqa[:], xt[:, p * D:(p + 1) * D], Act.Square,
                            accum_out=ss[:, p:p + 1])
                        if p == 0 and prev_sqrt is not None:
                            # pin ACT order: sqrt(i-1) must precede
                            # squares(i), else the scheduler makes r(i-1)
                            # wait on DMA(i)
                            add_dep_helper(
                                a.ins, prev_sqrt.ins, sync=False,
                                reason="sqrt(i-1) before squares(i)")
                    if dve_p3:
                        # tile 6: DVE absorbs part 3 so ACT is free for
                        # tile 7's parts the moment they land
                        p3 = NPARTS - 1
                        sq6 = scp.tile([TILE_P, D], bf16, tag="sq6")
                        nc.vector.tensor_mul(sq6[:], xt[:, p3 * D:],
                                             xt[:, p3 * D:])
                        nc.vector.tensor_reduce(ss[:, p3:p3 + 1], sq6[:],
                                                axis=AxX, op=Alu.add)
                    prev_sqrt = nc.scalar.sqrt(norm[:], ss[:])
                    rc = nc.vector.reciprocal(r[:], norm[:])
                    nc.vector.tensor_copy(r_bf[:], r[:])
                    for p in range(NPARTS):
                        mms_for_part(p, r_bf[:, p:p + 1])
                    if i == 1 and cp_a is not None:
                        # pin the S_a copy ahead of tile 1's normalize
                        # on the DVE queue so the AR1 doorbell fires at
                        # ~29us, not ~50us
                        add_dep_helper(rc.ins, cp_a.ins, sync=False,
                                       reason="S_a copy before recip(1)")
                else:
                    # tile 7: parts 0-2 on ACT (land while the stream
                    # still runs), part 3 in two DVE-squared chunks
                    ss3 = stp.tile([TILE_P, 3], f32, tag="ss3")
                    pa = None
                    for p in range(NPARTS - 1):
                        a = nc.scalar.activation(
                            sqa[:], xt[:, p * D:(p + 1) * D], Act.Square,
                            accum_out=ss[:, p:p + 1])
                        if p == 0 and prev_sqrt is not None:
                            add_dep_helper(a.ins, prev_sqrt.ins, sync=False,
                                           reason="sqrt(i-1) first")
                        if pa is not None:
                            add_dep_helper(a.ins, pa.ins, sync=False,
                                           reason="ACT part order")
                        pa = nc.scalar.sqrt(norm[:, p:p + 1], ss[:, p:p + 1])
                        nc.vector.reciprocal(r[:, p:p + 1], norm[:, p:p + 1])
                        nc.vector.tensor_copy(r_bf[:, p:p + 1], r[:, p:p + 1])
                        mms_for_part(p, r_bf[:, p:p + 1])
                    p3 = NPARTS - 1
                    sq3 = scp.tile([TILE_P, D], bf16, tag="sq3")
                    for c in range(2):
                        lo = p3 * D + c * (D // 2)
                        nc.vector.tensor_mul(
                            sq3[:, c * (D // 2):(c + 1) * (D // 2)],
                            xt[:, lo:lo + D // 2], xt[:, lo:lo + D // 2])
                        nc.vector.tensor_reduce(
                            ss3[:, c:c + 1],
                            sq3[:, c * (D // 2):(c + 1) * (D // 2)],
                            axis=AxX, op=Alu.add)
                    nc.vector.tensor_add(ss3[:, 2:3], ss3[:, 0:1],
                                         ss3[:, 1:2])
                    s3 = nc.scalar.sqrt(norm[:, p3:p3 + 1], ss3[:, 2:3])
                    if pa is not None:
                        add_dep_helper(s3.ins, pa.ins, sync=False,
                                       reason="ACT part order")
                    prev_sqrt = s3
                    nc.vector.reciprocal(r[:, p3:p3 + 1], norm[:, p3:p3 + 1])
                    nc.vector.tensor_copy(r_bf[:, p3:p3 + 1], r[:, p3:p3 + 1])
                    mms_for_part(p3, r_bf[:, p3:p3 + 1])

                if i == HALF - 1:
                    # first-tile partial sums: ship out + AllReduce now,
                    # overlapped with the rest of the DMA stream
                    s_sba = tlp.tile([TILE_P, D], f32, tag="s_sba")
                    cp_a = nc.vector.tensor_copy(s_sba[:], S_a[:])
                    for p in range(NPARTS):
                        nc.sync.dma_start(cc_in_a[p:p + 1, :],
                                          s_sba[32 * p:32 * p + 1, :])
                    if collective:
                        ar1 = nc.gpsimd.collective_compute(
                            "AllReduce", Alu.add,
                            replica_groups=[list(range(ncores))],
                            ins=[cc_in_a.opt()], outs=[cc_out_a.opt()])
                    else:
                        nc.sync.dma_start(cc_out_a[:], cc_in_a[:])

            # ---- AR1 tail precompute (idle window while AR2 is in
            # flight): load summed first tile as bf16, accumulate
            # sum_p via ones-matmuls into t_ps ----
            sfa = tlp.tile([NPARTS, D], bf16, tag="sfa")
            ld_a = nc.gpsimd.dma_start(sfa[:], cc_out_a[:])
            ones4 = tlp.tile([NPARTS, 1], bf16, tag="ones4")
            nc.vector.memset(ones4[:], 1.0)
            onesb = tlp.tile([NPARTS, 1], bf16, tag="onesb")
            nc.vector.memset(onesb[:], 1.0)
            # weights folding A - 3*B2 via two accumulating matmuls
            neg3 = tlp.tile([1, 1], f32, tag="neg3")
            nc.vector.memset(neg3[:], -3.0)
            one1 = tlp.tile([1, 1], f32, tag="one1")
            nc.vector.memset(one1[:], 1.0)
            g_sb = tlp.tile([1, 1], f32, tag="g_sb")
            nc.sync.dma_start(g_sb[:], g_t[:])

            # t = sum_p S_p accumulates in PSUM: sfa half now (start),
            # sfb half after AR2 (stop) — the sfa+sfb add is off the
            # t critical path entirely
            t_ps = psp.tile([1, D], f32, tag="accA")
            for j in range(NCHUNK):
                nc.tensor.matmul(
                    t_ps[0:1, j * MM_N:(j + 1) * MM_N],
                    lhsT=ones4[:],
                    rhs=sfa[:, j * MM_N:(j + 1) * MM_N],
                    start=True, stop=False)

            # ---- second-group partial sums -> AllReduce over 8 cores ----
            # one full-width PSUM->SBUF copy split across DVE and ACT
            # (rows besides 0/32/64/96 are junk zeros)
            s_sb = tlp.tile([TILE_P, D], f32, tag="s_sb")
            nc.vector.tensor_copy(s_sb[:, :D // 2], S_b[:, :D // 2])
            nc.scalar.copy(s_sb[:, D // 2:], S_b[:, D // 2:])

            for p in range(NPARTS):
                eng = nc.sync if p % 2 == 0 else nc.scalar
                eng.dma_start(cc_in_b[p:p + 1, :],
                              s_sb[32 * p:32 * p + 1, :])
            ar2 = None
            if collective:
                ar2 = nc.gpsimd.collective_compute(
                    "AllReduce", Alu.add,
                    replica_groups=[list(range(ncores))],
                    ins=[cc_in_b.opt()], outs=[cc_out_b.opt()])
                # keep gpsimd free to fire the AR2 doorbell before it
                # blocks on loading AR1's output
                add_dep_helper(ld_a.ins, ar1.ins, sync=False,
                               reason="AR1 done before sfa load")
            else:
                nc.sync.dma_start(cc_out_b[:], cc_in_b[:])

            sfb = tlp.tile([NPARTS, D], bf16, tag="sfb")
            ld_b = nc.gpsimd.dma_start(sfb[:], cc_out_b[:])
            if ar2 is not None:
                add_dep_helper(ld_b.ins, ar2.ins, sync=False,
                               reason="AR2 doorbell before sfb load")

            # ---- replicated tail: loss scalar ----
            for j in range(NCHUNK):
                nc.tensor.matmul(
                    t_ps[0:1, j * MM_N:(j + 1) * MM_N],
                    lhsT=ones4[:],
                    rhs=sfb[:, j * MM_N:(j + 1) * MM_N],
                    start=False, stop=True)

            # B2 = sum((sfa+sfb)^2): DVE add+mult, PE ones-matmuls
            # accumulate the part dim into a [1,512] PSUM vector, one
            # short ACT accumulate collapses it to a scalar
            sfull = tlp.tile([NPARTS, D], bf16, tag="sfull")
            nc.vector.tensor_add(sfull[:], sfa[:], sfb[:])
            sq_tail = tlp.tile([NPARTS, D], bf16, tag="sq_tail")
            nc.vector.tensor_mul(sq_tail[:], sfull[:], sfull[:])
            b2p_ps = psp.tile([1, MM_N], f32, tag="accB")
            for j in range(NCHUNK):
                nc.tensor.matmul(
                    b2p_ps[:], lhsT=onesb[:],
                    rhs=sq_tail[:, j * MM_N:(j + 1) * MM_N],
                    start=(j == 0), stop=(j == NCHUNK - 1))
            b2_sb = tlp.tile([1, 1], f32, tag="b2_sb")
            nc.vector.tensor_reduce(b2_sb[:], b2p_ps[:], axis=AxX,
                                    op=Alu.add)

            t_sq = tlp.tile([1, D], f32, tag="t_sq")
            a_sb = tlp.tile([1, 1], f32, tag="a_sb")
            nc.scalar.activation(t_sq[:], t_ps[:], Act.Square,
                                 accum_out=a_sb[:])

            ab_ps = psp.tile([1, 1], f32, tag="accB")
            nc.tensor.matmul(ab_ps[:], lhsT=one1[:], rhs=a_sb[:],
                             start=True, stop=False)
            nc.tensor.matmul(ab_ps[:], lhsT=neg3[:], rhs=b2_sb[:],
                             start=False, stop=True)

            # loss = ((A - 3*B2) / B^2 + 2P) / 2 * gamma
            l0 = tlp.tile([1, 1], f32, tag="l0")
            nc.vector.tensor_scalar(
                out=l0[:], in0=ab_ps[:],
                scalar1=1.0 / (2.0 * float(B) * float(B)),
                scalar2=float(NPARTS),
                op0=Alu.mult, op1=Alu.add)
            loss = tlp.tile([1, 1], f32, tag="loss")
            nc.vector.tensor_mul(loss[:], l0[:], g_sb[:])
            nc.sync.dma_start(out_t[:], loss[:])

    nc.compile()
    return nc


def _get_nc():
    if "nc" not in _cache:
        _cache["nc"] = _build()
    return _cache["nc"]


def kernel(x, gamma, **run_kwargs):
    from concourse import bass_utils

    x = np.ascontiguousarray(np.asarray(x, dtype=np.float32))
    gamma = np.asarray(gamma, dtype=np.float32).reshape(1, 1)
    assert x.shape == (B, F), x.shape

    nc = _get_nc()
    in_maps = [
        {"x": x[c * B_CORE:(c + 1) * B_CORE], "gamma": gamma}
        for c in range(NCORES)
    ]
    res = bass_utils.run_bass_kernel_spmd(
        nc, in_maps, core_ids=list(range(NCORES)), **run_kwargs)
    out = np.asarray(res.results[0]["out"], dtype=np.float32).reshape(1)
    if run_kwargs.get("trace"):
        _cache["last_results"] = res
    return out
